# revision 1
# baseline (speedup 1.0000x reference)
"""Trainium2 Bass kernel for nn_BatchTripletMarginLoss.

Math: loss = sum(relu(td + 1)) / n_valid over the B^3 triplet tensor, where
td[a,p,n] = (d[a,p] - d[a,n]) * valid[a,p,n].  Every invalid triplet
contributes exactly relu(margin) = 1.0, so
    loss = (hinge_sum + (B^3 - n_valid)) / n_valid
with hinge_sum = sum over valid (a,p,n) of relu(d_ap - d_an + 1).
n_valid and the invalid part are integer math on entity_types (host).
hinge_sum is computed on 8 NeuronCores.

Sharding: rows are permuted so each type is contiguous; anchors are grouped
by type into 16 row-chunks (<=64 rows), two chunks per core (partition
halves [0:64) and [64:128)).  Per core the device computes its anchors'
distance rows D [128,512] (PE matmul + relu/sqrt), the positive-pair bias
matrix B1 [128,SMAX] = d(anchor, s-th member of its own type) + 1 (pads
-1e6), and then SMAX fused hinge-reduce passes
    relu(B1[:,s] - D) summed over all 512 columns
split between the Scalar (ACT) and Vector (DVE) engines.  Same-type columns
of D are masked to ~2e4 by adding 4e8 to d^2 before sqrt, so they never
contribute.  All floating-point math on embeddings happens on device; the
host only does index bookkeeping on entity_types and the final scalar
combine.
"""
import os
import sys
import numpy as np

for p in ("/opt/trn_rl_repo", "/root/.axon_site/_ro/trn_rl_repo"):
    if p not in sys.path:
        sys.path.append(p)

B, D, NT = 512, 768, 16
MARGIN = 1.0
LARGE = float(2 ** 28)  # added to d^2 of same-type cols -> d = 2^14 >> bias
NEGBIG = -1.0e4          # bias value for padding slots/rows
KT = D // 128    # 6 contraction tiles

_cache = {}


def _patch_tile_drain():
    """The kernel-tail Drain emitted by TileContext carries one sync wait
    per engine/DMA-queue sem (11 here), but the TPB instruction encoding
    has a single wait slot and this walrus build refuses to split.  Patch
    _drain_and_barrier to pre-emit single-wait NOPs on SP and spread the
    drain's waits across them."""
    import concourse.mybir as mybir
    import concourse.tile as tile
    from concourse.vector_clock import ScopedClock

    if getattr(tile.TileContext, "_drain_split_patched", False):
        return

    def _drain_and_barrier(self, tick_clock, wait_clock):
        nops = [self.nc.sync.nop(nofuse=True) for _ in range(13)]
        drain_inst = self.nc.sync.drain()
        wait_clock.add_sem_waits(
            drain_inst.ins, ScopedClock({None: tick_clock.global_clock})
        )
        si = drain_inst.ins.sync_info
        waits = list(si.on_wait) if si and si.on_wait else []
        if len(waits) > 1:
            assert len(waits) - 1 <= len(nops), f"{len(waits)} drain waits"
            for w, nop in zip(waits[:-1], nops):
                old = nop.ins.sync_info
                upd = list(old.on_update) if old and old.on_update else []
                nop.ins.sync_info = mybir.SyncInfo(on_wait=[w], on_update=upd)
            drain_inst.ins.sync_info = mybir.SyncInfo(
                on_wait=[waits[-1]],
                on_update=list(si.on_update) if si.on_update else [],
            )
        self.nc.all_engine_barrier()
        assert self.sems is not None
        popped = self.nc._tile_sem_poison_stack.pop()
        assert popped is self._sem_poison
        self.nc.clear_and_free_semaphores(list(self.sems.allocated().values()))
        self.nc.all_engine_barrier()

    tile.TileContext._drain_and_barrier = _drain_and_barrier
    tile.TileContext._drain_split_patched = True


def _build_program(SMAX, n_act):
    import concourse.bass as bass
    import concourse.mybir as mybir
    import concourse.tile as tile

    _patch_tile_drain()
    fp32 = mybir.dt.float32
    bf16 = mybir.dt.bfloat16
    AF = mybir.ActivationFunctionType
    OP = mybir.AluOpType
    n_dve = SMAX - n_act

    nc = bass.Bass()
    d_rhsT = nc.declare_dram_parameter("rhsT", [D, B], fp32, isOutput=False)
    d_lhsT = nc.declare_dram_parameter("lhsT", [D, 128], fp32, isOutput=False)
    d_rhs2T = nc.declare_dram_parameter("rhs2T", [D, 2 * SMAX], fp32, isOutput=False)
    d_mrow = nc.declare_dram_parameter("maskrow2", [2, B], bf16, isOutput=False)
    d_fix = nc.declare_dram_parameter("fixmat", [128, SMAX], fp32, isOutput=False)
    d_out = nc.declare_dram_parameter("out", [128, 1], fp32, isOutput=True)

    with tile.TileContext(nc) as tc:
        with (
            tc.tile_pool(name="big", bufs=1) as big,
            tc.tile_pool(name="work", bufs=1) as work,
            tc.tile_pool(name="dum", bufs=8) as dum,
            tc.tile_pool(name="ps", bufs=1, space="PSUM") as ps,
        ):
            # ---- loads: gpsimd DMAs cast fp32->bf16 in flight; rhsT
            # split in two chunk tiles so squares start after chunk 1
            W2 = 2 * SMAX
            KH = KT // 2
            rhb1 = big.tile([128, KH * B], bf16, name="rhb1", tag="rhb1")
            rhb2 = big.tile([128, (KT - KH) * B], bf16, name="rhb2", tag="rhb2")
            lhb = big.tile([128, KT * 128], bf16, name="lhb", tag="lhb")
            r2ball = big.tile([128, KT * W2], bf16, name="r2ball", tag="r2ball")
            mrow0 = big.tile([1, B], bf16, name="mrow0", tag="mrow0")
            mrow1 = big.tile([1, B], bf16, name="mrow1", tag="mrow1")
            fixm = big.tile([128, SMAX], fp32, name="fixm", tag="fixm")
            nc.gpsimd.dma_start(
                rhb1[:].rearrange("p (k j) -> p k j", k=KH),
                d_rhsT[:KH * 128].rearrange("(k p) j -> p k j", p=128))
            nc.gpsimd.dma_start(
                lhb[:].rearrange("p (k j) -> p k j", k=KT),
                d_lhsT[:].rearrange("(k p) j -> p k j", p=128))
            nc.gpsimd.dma_start(
                r2ball[:].rearrange("p (k j) -> p k j", k=KT),
                d_rhs2T[:].rearrange("(k p) j -> p k j", p=128))
            nc.gpsimd.dma_start(
                rhb2[:].rearrange("p (k j) -> p k j", k=KT - KH),
                d_rhsT[KH * 128:].rearrange("(k p) j -> p k j", p=128))
            nc.sync.dma_start(mrow0[:], d_mrow[0:1, :])
            nc.sync.dma_start(mrow1[:], d_mrow[1:2, :])
            nc.sync.dma_start(fixm[:], d_fix[:])
            rhb = [rhb1[:, k * B:(k + 1) * B] for k in range(KH)] + \
                  [rhb2[:, k * B:(k + 1) * B] for k in range(KT - KH)]
            lhbk = [lhb[:, k * 128:(k + 1) * 128] for k in range(KT)]
            r2b = [r2ball[:, k * W2:(k + 1) * W2] for k in range(KT)]

            ones128 = work.tile([128, 128], bf16, name="ones128", tag="ones128")
            ones_cb = work.tile([128, 1], bf16, name="ones_cb", tag="ones_cb")
            ones_rb = work.tile([1, 64], bf16, name="ones_rb", tag="ones_rb")
            ones_row = work.tile([1, B], bf16, name="ones_row", tag="ones_row")
            one_f = work.tile([1, 1], fp32, name="one_f", tag="one_f")
            nc.vector.memset(ones128[:], 1.0)
            nc.vector.memset(ones_cb[:], 1.0)
            nc.vector.memset(ones_rb[:], 1.0)
            nc.vector.memset(ones_row[:], 1.0)
            nc.vector.memset(one_f[:], 1.0)

            # DVE probe absorbing the fixm DMA dep early
            probe = work.tile([1, 2], fp32, name="probe", tag="probe")
            nc.vector.tensor_copy(probe[:1, 1:2], fixm[:1, 0:1])

            # PE absorbs for DMA-fed matmul operands (LDWEIGHTS: 1 wait slot)
            p_tr = ps.tile([1, 1], fp32, name="p_tr", tag="p_tr")
            nc.tensor.matmul(p_tr[:], mrow0[:, 0:1], mrow0[:, 0:1],
                             start=True, stop=True)
            nc.tensor.matmul(p_tr[:], mrow1[:, 0:1], mrow1[:, 0:1],
                             start=True, stop=True)

            # ---- per-k front: casts, squares, then PE accumulations ----
            # p_D accumulates: -2*G (bf16) + ones128^T@sqt (broadcast sum of
            # per-dim squares = sqX) + LARGE mask rows.  All bf16 matmuls.
            p_D = ps.tile([128, B], fp32, name="p_D", tag="p_D")
            p_B = ps.tile([128, SMAX], fp32, name="p_B", tag="p_B")
            p_sqA = ps.tile([1, 128], fp32, name="p_sqA", tag="p_sqA")
            p_sq2 = ps.tile([1, 2 * SMAX], fp32, name="p_sq2", tag="p_sq2")

            lhm2 = []
            for k in range(KT):
                # ACT: prescale lhsT -> -2x (bf16, exact exponent shift)
                m2 = work.tile([128, 128], bf16, name=f"lhm2_{k}", tag=f"lhm2_{k}")
                nc.scalar.mul(m2[:], lhbk[k][:], -2.0)
                lhm2.append(m2)
                # DVE: squares
                sqt = dum.tile([128, B], bf16, name="sqt", tag="sqt")
                nc.vector.tensor_mul(sqt[:], rhb[k][:], rhb[k][:])
                r2sq = dum.tile([128, 2 * SMAX], bf16, name="r2sq", tag="r2sq")
                nc.vector.tensor_mul(r2sq[:], r2b[k][:], r2b[k][:])
                sqa = dum.tile([128, 128], bf16, name="sqa", tag="sqa")
                nc.vector.tensor_mul(sqa[:], m2[:], m2[:])  # 4x^2, /4 later

                # PE: absorbs (1-wait LDWEIGHTS rule), then accumulations
                nc.tensor.matmul(p_tr[:], m2[:, 0:1], m2[:, 0:1],
                                 start=True, stop=True)
                nc.tensor.matmul(p_tr[:], rhb[k][:, 0:1], rhb[k][:, 0:1],
                                 start=True, stop=True)
                nc.tensor.matmul(p_tr[:], r2b[k][:, 0:1], r2b[k][:, 0:1],
                                 start=True, stop=True)
                nc.tensor.matmul(p_D[:], ones128[:], sqt[:],
                                 start=(k == 0), stop=False)
                nc.tensor.matmul(p_D[:], m2[:], rhb[k][:],
                                 start=False, stop=False)
                nc.tensor.matmul(p_sq2[:], ones_cb[:], r2sq[:],
                                 start=(k == 0), stop=(k == KT - 1))
                nc.tensor.matmul(p_sqA[:], ones_cb[:], sqa[:],
                                 start=(k == 0), stop=(k == KT - 1))
                nc.tensor.matmul(p_B[0:64, :], m2[:, 0:64], r2b[k][:, 0:SMAX],
                                 start=(k == 0), stop=False)
                nc.tensor.matmul(p_B[64:128, :], m2[:, 64:128],
                                 r2b[k][:, SMAX:2 * SMAX],
                                 start=(k == 0), stop=False,
                                 skip_group_check=True)

            # mask rows into p_D halves (bf16 k=1 matmuls)
            nc.tensor.matmul(p_D[0:64, :], ones_rb[:], mrow0[:],
                             start=False, stop=False, skip_group_check=True)
            nc.tensor.matmul(p_D[64:128, :], ones_rb[:], mrow1[:],
                             start=False, stop=False, skip_group_check=True)

            # sq2 broadcast rows into p_B halves
            sq2_r = work.tile([1, 2 * SMAX], bf16, name="sq2_r", tag="sq2_r")
            nc.vector.tensor_copy(sq2_r[:], p_sq2[:])
            nc.tensor.matmul(p_B[0:64, :], ones_rb[:], sq2_r[:, 0:SMAX],
                             start=False, stop=False, skip_group_check=True)
            nc.tensor.matmul(p_B[64:128, :], ones_rb[:],
                             sq2_r[:, SMAX:2 * SMAX],
                             start=False, stop=False, skip_group_check=True)

            # sqA as a bf16 row; folded into p_D / p_B via rank-1 matmuls
            sqA_rb = work.tile([1, 128], bf16, name="sqA_rb", tag="sqA_rb")
            nc.vector.tensor_scalar_mul(sqA_rb[:], p_sqA[:], 0.25)
            nc.tensor.matmul(p_D[:], sqA_rb[:], ones_row[:],
                             start=False, stop=True, skip_group_check=True)
            nc.tensor.matmul(p_B[:], sqA_rb[:], ones_row[:, 0:SMAX],
                             start=False, stop=True, skip_group_check=True)

            # ---- D rows: masked cols carry +2^28 -> sqrt ~ 2^14 ----
            Dfull = big.tile([128, B], bf16, name="Dfull", tag="Dfull")
            nc.scalar.activation(Dfull[:], p_D[:], AF.Sqrt)

            # ---- B1 = d(anchor, own-type member s) + 1 (pads -> NEGBIG) ----
            Bsq = work.tile([128, SMAX], fp32, name="Bsq", tag="Bsq")
            nc.scalar.activation(Bsq[:], p_B[:], AF.Relu)
            Bs = work.tile([128, SMAX], fp32, name="Bs", tag="Bs")
            nc.scalar.activation(Bs[:], Bsq[:], AF.Sqrt)
            B1 = work.tile([128, SMAX], fp32, name="B1", tag="B1")
            nc.vector.tensor_add(B1[:], Bs[:], fixm[:])

            # ---- hinge: ACT slots [0,n_act) relu-sum; DVE slots use the
            # identity sum_j relu(b-d_j) = 512*b - sum_j min(d_j, b)
            hacc = work.tile([128, max(n_act, 1)], fp32, name="hacc", tag="hacc")
            dacc = work.tile([128, max(n_dve, 1)], fp32, name="dacc", tag="dacc")
            if n_act == 0:
                nc.vector.memset(hacc[:], 0.0)
            if n_dve == 0:
                nc.vector.memset(dacc[:], 0.0)
            for s in range(n_act):
                dumA = dum.tile([128, B], bf16, name="dumA", tag="dumA")
                nc.scalar.activation(
                    dumA[:], Dfull[:], AF.Relu,
                    bias=B1[:, s:s + 1], scale=-1.0,
                    accum_out=hacc[:, s:s + 1])
            for i, s in enumerate(range(n_act, SMAX)):
                dumD = dum.tile([128, B], bf16, name="dumD", tag="dumD")
                nc.vector.tensor_scalar(
                    dumD[:], Dfull[:], B1[:, s:s + 1], None,
                    op0=OP.min, op1=OP.add,
                    accum_out=dacc[:, i:i + 1])

            # ---- tail: res = sum(hacc) + 512*rowsum(B1 dve cols) - sum(dacc)
            h1 = work.tile([128, 1], fp32, name="h1", tag="h1")
            d1 = work.tile([128, 1], fp32, name="d1", tag="d1")
            b1s = work.tile([128, 1], fp32, name="b1s", tag="b1s")
            nc.vector.tensor_reduce(h1[:], hacc[:], axis=mybir.AxisListType.X,
                                    op=OP.add)
            nc.vector.tensor_reduce(d1[:], dacc[:], axis=mybir.AxisListType.X,
                                    op=OP.add)
            if n_dve > 0:
                nc.vector.tensor_reduce(b1s[:], B1[:, n_act:SMAX],
                                        axis=mybir.AxisListType.X, op=OP.add)
            else:
                nc.vector.memset(b1s[:], 0.0)
            t1 = work.tile([128, 1], fp32, name="t1", tag="t1")
            nc.vector.tensor_scalar(t1[:], b1s[:], float(B), None, op0=OP.mult)
            t2 = work.tile([128, 1], fp32, name="t2", tag="t2")
            nc.vector.tensor_sub(t2[:], t1[:], d1[:])
            res = work.tile([128, 1], fp32, name="res", tag="res")
            nc.vector.tensor_add(res[:], h1[:], t2[:])
            nc.scalar.dma_start(d_out[:], res[:])

    return nc


def _host_prep(entity_types, embeddings):
    t = np.asarray(entity_types).astype(np.int64)
    X = np.ascontiguousarray(np.asarray(embeddings), dtype=np.float32)
    counts = np.bincount(t, minlength=NT)
    assert counts.sum() == B
    perm = np.argsort(t, kind="stable")
    offs = np.zeros(NT + 1, dtype=np.int64)
    offs[1:] = np.cumsum(counts)
    Xp = X[perm]

    items = []
    for k in range(NT):
        s, e = int(offs[k]), int(offs[k + 1])
        r = s
        while r < e:
            items.append((k, r, min(r + 64, e)))
            r = min(r + 64, e)
    assert len(items) <= 16, f"too many row-chunks: {len(items)}"
    while len(items) < 16:
        items.append((-1, 0, 0))
    cost = lambda it: (it[2] - it[1]) * (counts[it[0]] if it[0] >= 0 else 0)
    items.sort(key=cost, reverse=True)
    pairs = [(items[c], items[15 - c]) for c in range(8)]

    SMAX = int(counts.max())
    rhsT = np.ascontiguousarray(Xp.T)

    in_maps = []
    for itA, itB in pairs:
        lhsT = np.zeros((D, 128), np.float32)
        import ml_dtypes
        maskrow2 = np.zeros((2, B), ml_dtypes.bfloat16)
        rhs2T = np.zeros((D, 2 * SMAX), np.float32)
        fixmat = np.full((128, SMAX), NEGBIG, np.float32)
        for h, (k, rs, re) in enumerate((itA, itB)):
            if k < 0 or rs >= re:
                continue
            nrow = re - rs
            ck = int(counts[k])
            ts_, te = int(offs[k]), int(offs[k + 1])
            lhsT[:, 64 * h:64 * h + nrow] = Xp[rs:re].T
            maskrow2[h, ts_:te] = LARGE
            rhs2T[:, SMAX * h:SMAX * h + ck] = Xp[ts_:te].T
            fixmat[64 * h:64 * h + nrow, :ck] = MARGIN
        in_maps.append(dict(rhsT=rhsT, lhsT=lhsT, rhs2T=rhs2T,
                            maskrow2=maskrow2, fixmat=fixmat))

    n_valid = int((counts.astype(np.int64) ** 2 * (B - counts)).sum())
    return in_maps, SMAX, n_valid


def kernel(entity_types, embeddings):
    from concourse.bass_utils import run_bass_kernel_spmd

    in_maps, SMAX, n_valid = _host_prep(entity_types, embeddings)
    n_act = int(os.environ.get("N_ACT", max(1, round(SMAX * 0.33))))
    key = (SMAX, n_act)
    if key not in _cache:
        _cache[key] = _build_program(SMAX, n_act)
    nc = _cache[key]

    r = run_bass_kernel_spmd(nc, in_maps, core_ids=list(range(8)))
    hinge = 0.0
    for c in range(8):
        hinge += np.asarray(r.results[c]["out"], dtype=np.float64).sum()
    total = hinge + MARGIN * (B ** 3 - n_valid)
    return np.asarray(np.float32(total / n_valid))

